# revision 11
# baseline (speedup 1.0000x reference)
"""Trainium2 Bass kernel for nn_ChainPOSAwareLM (T=64, B=32, vocab 32000).

Architecture (8 NeuronCores, full I/O):
- Phase B (64-step coupled LSTM recurrence) runs REPLICATED on every core:
  per-step cross-core sync is impossible to do fast with this toolchain
  (chained ncfw collectives measure ~10-15us; remote_dma is rejected by this
  walrus build), and the ~16M recurrent weight elements cannot be re-streamed
  from HBM per step. All recurrent matmuls keep batch(32) stationary (hT as
  lhsT, produced via PE transposes) with the weight as the moving operand,
  4x column-tile packed, so the PE ingests 4*128 weight elems/cycle.
- Five large matrices are stored fp8-e4m3 so the recurrent set fits SBUF;
  psum accumulation is fp32. (numpy sim: ~3e-3 scale-relative output error.)
- h1/p_h histories stream to DRAM; phase C reloads them per 128-token m-tile.
- Phase C (tied projection + log_softmax over 32000) is vocab-sharded
  (4000/core): matmul -> exp with fused row-sum (accum_out) -> one small
  AllReduce of Z per m-tile -> out = logits - ln(Z). Pos head replicated.
"""
import numpy as np
import ml_dtypes

import concourse.bass as bass
import concourse.mybir as mybir
from concourse.tile import TileContext
from concourse.bass_utils import run_bass_kernel_spmd

BF16 = ml_dtypes.bfloat16
E4M3 = ml_dtypes.float8_e4m3fn

F32 = mybir.dt.float32
BF = mybir.dt.bfloat16
F8 = mybir.dt.float8e4
AF = mybir.ActivationFunctionType
ALU = mybir.AluOpType

T, B = 64, 32
POS_V, WORD_V = 48, 32000
P_EMB, W_EMB = 128, 512
P_HID, W_HID = 256, 1024
NC = 8
VS = WORD_V // NC
NVC = 8
VCW = VS // NVC


def _reord(w):
    i, f, g, o = np.split(w, 4, axis=0)
    return np.concatenate([i, f, o, g], axis=0)


def _host_prep(inp):
    P = {k: np.asarray(v, np.float32) for k, v in inp.items()
         if k not in ("pos", "word")}
    pos = np.asarray(inp["pos"]).astype(np.int64)
    word = np.asarray(inp["word"]).astype(np.int64)
    p_emb = P["pos_emb_W"][pos]
    w_emb = P["word_emb_W"][word]
    pre_pos = (p_emb @ _reord(P["p_Wih0"])[:, :P_EMB].T
               + _reord(P["p_bih0"][:, None])[:, 0]
               + _reord(P["p_bhh0"][:, None])[:, 0])
    pre_w0 = (w_emb @ _reord(P["w_Wih0"])[:, :W_EMB].T
              + _reord(P["w_bih0"][:, None])[:, 0]
              + _reord(P["w_bhh0"][:, None])[:, 0])
    b1 = (_reord(P["w_bih1"][:, None])[:, 0]
          + _reord(P["w_bhh1"][:, None])[:, 0])

    d = {}
    d["pre_pos"] = pre_pos.astype(BF16)                    # [T,B,1024]
    d["pre_w0"] = pre_w0.astype(BF16)                      # [T,B,4096]
    d["w2p_rhs"] = P["w2p_W"].T.astype(BF16)
    d["pwih_lw"] = _reord(P["p_Wih0"])[:, P_EMB:].T.astype(E4M3)
    d["pwhh"] = _reord(P["p_Whh0"]).T.astype(BF16)
    d["p2w_rhs"] = P["p2w_W"].T.astype(BF16)
    d["wih0_lp"] = _reord(P["w_Wih0"])[:, W_EMB:].T.astype(E4M3)
    d["whh0"] = _reord(P["w_Whh0"]).T.astype(E4M3)
    d["wih1"] = _reord(P["w_Wih1"]).T.astype(E4M3)
    d["whh1"] = _reord(P["w_Whh1"]).T.astype(E4M3)
    d["proj1_rhs"] = P["word_proj1_W"].T.astype(BF16)
    d["posproj_rhs"] = P["pos_proj_W"].T.astype(BF16)
    d["ident32"] = np.eye(32, dtype=np.float32)
    d["ident128"] = np.eye(128, dtype=np.float32)
    d["ones1"] = np.ones((1, 32), BF16)
    d["ones128r"] = np.ones((1, 128), BF16)
    # biases that are not folded into precomps
    d["w2p_b_row"] = P["w2p_b"][None, :].astype(BF16)
    d["p2w_b_row"] = P["p2w_b"][None, :].astype(BF16)
    d["b1_row"] = b1[None, :].astype(BF16)
    d["proj1_b_row"] = P["word_proj1_b"][None, :].astype(BF16)
    d["posproj_b_row"] = P["pos_proj_b"][None, :].astype(BF16)
    nz = {
        "w2p_b_row": np.abs(P["w2p_b"]).max() > 0,
        "p2w_b_row": np.abs(P["p2w_b"]).max() > 0,
        "b1_row": np.abs(b1).max() > 0,
        "proj1_b_row": np.abs(P["word_proj1_b"]).max() > 0,
        "posproj_b_row": np.abs(P["pos_proj_b"]).max() > 0,
        "b2_row": np.abs(P["word_proj2_b"]).max() > 0,
    }
    embT = P["word_emb_W"].T.astype(BF16)
    b2 = P["word_proj2_b"].astype(BF16)
    percore = []
    for c in range(NC):
        percore.append({
            "embT_sl": np.ascontiguousarray(embT[:, c * VS:(c + 1) * VS]),
            "b2_row": np.ascontiguousarray(b2[None, c * VS:(c + 1) * VS]),
        })
    return d, percore, nz


def build(nsteps, nz):
    nc = bass.Bass()
    q = {}

    def par(name, shape, dt):
        q[name] = nc.declare_dram_parameter(name, list(shape), dt, isOutput=False)

    par("pre_pos", (T, B, 4 * P_HID), BF)
    par("pre_w0", (T, B, 4 * W_HID), BF)
    par("w2p_rhs", (W_HID, W_HID), BF)
    par("pwih_lw", (W_HID, 4 * P_HID), F8)
    par("pwhh", (P_HID, 4 * P_HID), BF)
    par("p2w_rhs", (P_HID, P_HID), BF)
    par("wih0_lp", (P_HID, 4 * W_HID), F8)
    par("whh0", (W_HID, 4 * W_HID), F8)
    par("wih1", (W_HID, 4 * W_HID), F8)
    par("whh1", (W_HID, 4 * W_HID), F8)
    par("proj1_rhs", (W_HID, W_EMB), BF)
    par("posproj_rhs", (P_HID, POS_V), BF)
    par("ident32", (32, 32), F32)
    par("ident128", (128, 128), F32)
    par("ones1", (1, 32), BF)
    par("ones128r", (1, 128), BF)
    par("w2p_b_row", (1, W_HID), BF)
    par("p2w_b_row", (1, P_HID), BF)
    par("b1_row", (1, 4 * W_HID), BF)
    par("proj1_b_row", (1, W_EMB), BF)
    par("posproj_b_row", (1, POS_V), BF)
    par("embT_sl", (W_EMB, VS), BF)
    par("b2_row", (1, VS), BF)

    ntok = nsteps * B
    nmt = ntok // 128
    w_out = nc.declare_dram_parameter("w_out", [ntok, VS], F32, isOutput=True)
    p_out = nc.declare_dram_parameter("p_out", [ntok, POS_V], F32, isOutput=True)

    h1_dram = nc.dram_tensor("h1_hist", [128, nsteps, 8, 32], BF)
    ph_dram = nc.dram_tensor("ph_hist", [128, nsteps, 2, 32], BF)
    zin = [nc.dram_tensor(f"zin{m}", [128, 1], F32) for m in range(nmt)]
    zout = [nc.dram_tensor(f"zout{m}", [128, 1], F32, addr_space="Shared")
            for m in range(nmt)]
    cores = list(range(NC))

    with TileContext(nc) as tc:
        with tc.tile_pool(name="const", bufs=1) as cst:
            def loadc(name):
                src = q[name]
                t_ = cst.tile(list(src.shape), src.dtype, tag=name)
                nc.sync.dma_start(out=t_[:], in_=src[:])
                return t_

            id32 = loadc("ident32")
            id128 = loadc("ident128")
            ones1 = loadc("ones1")
            ones128 = loadc("ones128r")

            # ===================== PHASE B =====================
            with (
                tc.tile_pool(name="bw", bufs=1) as wp,
                tc.tile_pool(name="bstate", bufs=1) as sp,
                tc.tile_pool(name="bwork", bufs=1) as kp,
                tc.tile_pool(name="bpipe1", bufs=1) as dp1,
                tc.tile_pool(name="bps", bufs=1, space="PSUM") as ps,
                tc.tile_pool(name="bpsg", bufs=2, space="PSUM") as psg,
                tc.tile_pool(name="bpsT", bufs=2, space="PSUM") as psT,
            ):
                def load_kn(name, K, N):
                    src = q[name]
                    nk = K // 128
                    t_ = wp.tile([128, nk * N], src.dtype, tag=name)
                    for k in range(nk):
                        nc.sync.dma_start(out=t_[:, k * N:(k + 1) * N],
                                          in_=src[k * 128:(k + 1) * 128, :])
                    return t_

                id32b = cst.tile([32, 32], BF, tag="id32b")
                nc.vector.tensor_copy(id32b[:], id32[:])
                w2p_w = load_kn("w2p_rhs", W_HID, W_HID)
                pwihlw_w = load_kn("pwih_lw", W_HID, 4 * P_HID)
                pwhh_w = load_kn("pwhh", P_HID, 4 * P_HID)
                p2w_w = load_kn("p2w_rhs", P_HID, P_HID)
                wih0lp_w = load_kn("wih0_lp", P_HID, 4 * W_HID)
                whh0_w = load_kn("whh0", W_HID, 4 * W_HID)
                wih1_w = load_kn("wih1", W_HID, 4 * W_HID)
                whh1_w = load_kn("whh1", W_HID, 4 * W_HID)
                w2pb = loadc("w2p_b_row") if nz["w2p_b_row"] else None
                p2wb = loadc("p2w_b_row") if nz["p2w_b_row"] else None
                b1r = loadc("b1_row") if nz["b1_row"] else None

                # state windows (bf16 transposed layout), double buffered
                h1w = [sp.tile([128, 8 * 32], BF, tag=f"h1w{j}", name=f"h1w{j}") for j in range(2)]
                phw = [sp.tile([128, 2 * 32], BF, tag=f"phw{j}", name=f"phw{j}") for j in range(2)]
                h0w = [sp.tile([128, 8 * 32], BF, tag=f"h0w{j}", name=f"h0w{j}") for j in range(2)]
                nc.vector.memset(h1w[1][:], 0.0)
                nc.vector.memset(phw[1][:], 0.0)
                nc.vector.memset(h0w[1][:], 0.0)
                c_pos = sp.tile([64, P_HID], F32, tag="c_pos")
                c_w0 = sp.tile([64, W_HID], F32, tag="c_w0")
                c_w1 = sp.tile([64, W_HID], F32, tag="c_w1")
                nc.vector.memset(c_pos[32:64, :], 0.0)
                nc.vector.memset(c_w0[32:64, :], 0.0)
                nc.vector.memset(c_w1[32:64, :], 0.0)

                def packed(psum, nper, kblocks, pre=None, bias=None):
                    """col-packed gate accumulation into psum [128, nper].
                    kblocks: (lhsT_ap, w_tile, kidx, wN). Issue order is
                    (sub, k, cg) so 4 consecutive matmuls hit the 4 column
                    groups and stream concurrently."""
                    nsub = (nper + 511) // 512
                    ops = list(kblocks)
                    extra = []
                    if bias is not None:
                        extra.append(("bias", bias))
                    if pre is not None:
                        extra.append(("pre", pre))
                    nops = len(ops) + len(extra)
                    for sub in range(nsub):
                        c0 = sub * 512
                        cw = min(512, nper - c0)
                        for i in range(nops):
                            for cg in range(4):
                                base = cg * nper + c0
                                if i < len(ops):
                                    lhsT, w_t, kidx, wN = ops[i]
                                    l_ = lhsT
                                    r_ = w_t[:, kidx * wN + base:
                                             kidx * wN + base + cw]
                                else:
                                    kind, tile_ = extra[i - len(ops)]
                                    l_ = ones1[:] if kind == "bias" else id32b[:]
                                    r_ = tile_[:, base:base + cw]
                                nc.tensor.matmul(
                                    psum[cg * 32:(cg + 1) * 32, c0:c0 + cw],
                                    l_, r_,
                                    start=(i == 0), stop=(i == nops - 1),
                                    tile_position=(0, cg * 32))


                def cell(psum, nper, c_tile, hout, cellid=""):
                    sifo = kp.tile([96, nper], BF, tag=f"sifo{cellid}",
                                   name=f"sifo{cellid}")
                    tgtc = kp.tile([96, nper], BF, tag=f"tgtc{cellid}",
                                   name=f"tgtc{cellid}")
                    t1 = kp.tile([64, nper], BF, tag=f"t1{cellid}",
                                 name=f"t1{cellid}")
                    t2 = kp.tile([64, nper], F32, tag=f"t2{cellid}",
                                 name=f"t2{cellid}")
                    nc.scalar.activation(out=sifo[0:96, :nper],
                                         in_=psum[0:96, :nper], func=AF.Sigmoid)
                    nc.gpsimd.tensor_tensor(out=t2[32:64, :nper],
                                            in0=sifo[32:64, :nper],
                                            in1=c_tile[32:64, :nper], op=ALU.mult)
                    nc.scalar.activation(out=tgtc[0:32, :nper],
                                         in_=psum[96:128, :nper], func=AF.Tanh)
                    nc.vector.tensor_tensor(out=t1[32:64, :nper],
                                            in0=sifo[0:32, :nper],
                                            in1=tgtc[0:32, :nper], op=ALU.mult)
                    nc.vector.tensor_tensor(out=c_tile[32:64, :nper],
                                            in0=t1[32:64, :nper],
                                            in1=t2[32:64, :nper], op=ALU.add)
                    nc.scalar.activation(out=tgtc[64:96, :nper],
                                         in_=c_tile[32:64, :nper], func=AF.Tanh)
                    nc.vector.tensor_tensor(out=hout[0:32, :nper],
                                            in0=sifo[64:96, :nper],
                                            in1=tgtc[64:96, :nper], op=ALU.mult)

                def transp(h_sbuf, nper, dst, dst_base):
                    """h_sbuf [32, nper] bf16 -> bf16 transposed into
                    dst[:, dst_base + k*32 ...]."""
                    for k in range(nper // 128):
                        pt = psT.tile([128, 32], BF, tag="tr")
                        nc.tensor.transpose(
                            out=pt[:], in_=h_sbuf[0:32, k * 128:(k + 1) * 128],
                            identity=id32b[:])
                        nc.vector.tensor_copy(
                            dst[:, dst_base + k * 32:dst_base + (k + 1) * 32],
                            pt[:])

                for t in range(nsteps):
                    cur, prv = t % 2, (t + 1) % 2
                    # ---- last_w = tanh(w2p @ h1_prev (+b)) ----
                    pw = psg.tile([128, 256], F32, tag="pgate")
                    packed(pw, 256,
                           [(h1w[prv][:, k * 32:(k + 1) * 32], w2p_w, k, W_HID)
                            for k in range(8)], bias=w2pb)
                    lastw = kp.tile([32, W_HID], BF, tag="lastw")
                    for cg in range(4):
                        nc.scalar.activation(
                            out=lastw[0:32, cg * 256:(cg + 1) * 256],
                            in_=pw[cg * 32:(cg + 1) * 32, 0:256], func=AF.Tanh)
                    lwT = kp.tile([128, 8 * 32], BF, tag="lwT")
                    transp(lastw, W_HID, lwT, 0)

                    # ---- pos gates / cell ----
                    pre_p = dp1.tile([32, 4 * P_HID], BF, tag="pre_p")
                    nc.sync.dma_start(out=pre_p[:], in_=q["pre_pos"][t])
                    pg = psg.tile([128, 256], F32, tag="pgate")
                    packed(pg, 256,
                           [(lwT[:, k * 32:(k + 1) * 32], pwihlw_w, k, 4 * P_HID)
                            for k in range(8)]
                           + [(phw[prv][:, k * 32:(k + 1) * 32], pwhh_w, k,
                               4 * P_HID) for k in range(2)],
                           pre=pre_p)
                    ph_new = kp.tile([32, P_HID], BF, tag="phnew")
                    cell(pg, P_HID, c_pos, ph_new, "p")
                    transp(ph_new, P_HID, phw[cur], 0)
                    nc.sync.dma_start(out=ph_dram[:, t], in_=phw[cur][:])

                    # ---- last_p = tanh(p2w @ p_h (+b)) ----
                    pp = psg.tile([32, P_HID], F32, tag="pgate")
                    lp_ops = [(phw[cur][:, k * 32:(k + 1) * 32],
                               p2w_w[:, k * P_HID:(k + 1) * P_HID])
                              for k in range(2)]
                    if p2wb is not None:
                        lp_ops.append((ones1[:], p2wb[:]))
                    for i, (l_, r_) in enumerate(lp_ops):
                        nc.tensor.matmul(pp[:, 0:P_HID], l_, r_,
                                         start=(i == 0),
                                         stop=(i == len(lp_ops) - 1))
                    lastp = kp.tile([32, P_HID], BF, tag="lastp")
                    nc.scalar.activation(out=lastp[:], in_=pp[:], func=AF.Tanh)
                    lpT = kp.tile([128, 2 * 32], BF, tag="lpT")
                    transp(lastp, P_HID, lpT, 0)

                    # ---- word LSTM 0 ----
                    pre0 = dp1.tile([32, 4 * W_HID], BF, tag="pre0")
                    nc.sync.dma_start(out=pre0[:], in_=q["pre_w0"][t])
                    g0 = ps.tile([128, W_HID], F32, tag="g0")
                    packed(g0, W_HID,
                           [(lpT[:, k * 32:(k + 1) * 32], wih0lp_w, k,
                             4 * W_HID) for k in range(2)]
                           + [(h0w[prv][:, k * 32:(k + 1) * 32], whh0_w, k,
                               4 * W_HID) for k in range(8)],
                           pre=pre0)
                    h0_new = kp.tile([32, W_HID], BF, tag="h0new")
                    cell(g0, W_HID, c_w0, h0_new, "0")
                    transp(h0_new, W_HID, h0w[cur], 0)

                    # ---- word LSTM 1 ----
                    g1 = ps.tile([128, W_HID], F32, tag="g1")
                    packed(g1, W_HID,
                           [(h0w[cur][:, k * 32:(k + 1) * 32], wih1_w, k,
                             4 * W_HID) for k in range(8)]
                           + [(h1w[prv][:, k * 32:(k + 1) * 32], whh1_w, k,
                               4 * W_HID) for k in range(8)],
                           bias=b1r)
                    h1_new = kp.tile([32, W_HID], BF, tag="h1new")
                    cell(g1, W_HID, c_w1, h1_new, "1")
                    transp(h1_new, W_HID, h1w[cur], 0)
                    nc.sync.dma_start(out=h1_dram[:, t], in_=h1w[cur][:])

            # ===================== PHASE C =====================
            with (
                tc.tile_pool(name="cw", bufs=1) as cw,
                tc.tile_pool(name="cwork", bufs=2) as cp,
                tc.tile_pool(name="cstage", bufs=3) as stp,
                tc.tile_pool(name="cps", bufs=1, space="PSUM") as ps2,
                tc.tile_pool(name="cpsv", bufs=2, space="PSUM") as psv,
                tc.tile_pool(name="cpsT", bufs=2, space="PSUM") as psT2,
            ):
                def load_kn2(name, K, N):
                    src = q[name]
                    nk = K // 128
                    t_ = cw.tile([128, nk * N], src.dtype, tag=name)
                    for k in range(nk):
                        nc.sync.dma_start(out=t_[:, k * N:(k + 1) * N],
                                          in_=src[k * 128:(k + 1) * 128, :])
                    return t_

                proj1_w = load_kn2("proj1_rhs", W_HID, W_EMB)
                posproj_w = load_kn2("posproj_rhs", P_HID, POS_V)
                embT_w = load_kn2("embT_sl", W_EMB, VS)
                proj1b = loadc("proj1_b_row") if nz["proj1_b_row"] else None
                posprojb = loadc("posproj_b_row") if nz["posproj_b_row"] else None
                b2r = loadc("b2_row") if nz["b2_row"] else None

                stags = {}
                lses = {}

                def c_compute(m):
                    tok0 = m * 128
                    s0 = tok0 // 32
                    h1m = [cp.tile([128, 128], BF, tag=f"h1m{k}", name=f"h1m{k}")
                           for k in range(8)]
                    for k in range(8):
                        nc.sync.dma_start(
                            out=h1m[k].rearrange("p (s c) -> p s c", s=4),
                            in_=h1_dram[:, s0:s0 + 4, k, :])
                    phm = [cp.tile([128, 128], BF, tag=f"phm{k}", name=f"phm{k}")
                           for k in range(2)]
                    for k in range(2):
                        nc.sync.dma_start(
                            out=phm[k].rearrange("p (s c) -> p s c", s=4),
                            in_=ph_dram[:, s0:s0 + 4, k, :])

                    # pos head
                    pl = ps2.tile([128, POS_V], F32, tag="pl")
                    pl_ops = [(phm[k][:], posproj_w[:, k * POS_V:(k + 1) * POS_V])
                              for k in range(2)]
                    if posprojb is not None:
                        pl_ops.append((ones128[:], posprojb[:]))
                    for i, (l_, r_) in enumerate(pl_ops):
                        nc.tensor.matmul(pl[:, 0:POS_V], l_, r_, start=(i == 0),
                                         stop=(i == len(pl_ops) - 1))
                    pexp = cp.tile([128, POS_V], F32, tag="pexp")
                    pzs = cp.tile([128, 1], F32, tag="pzs")
                    nc.scalar.activation(out=pexp[:], in_=pl[:], func=AF.Exp,
                                         accum_out=pzs[:])
                    plse = cp.tile([128, 1], F32, tag="plse")
                    nc.scalar.activation(out=plse[:], in_=pzs[:], func=AF.Ln)
                    pout_t = cp.tile([128, POS_V], F32, tag="pout_t")
                    nc.vector.tensor_scalar(out=pout_t[:], in0=pl[:],
                                            scalar1=plse[:], scalar2=None,
                                            op0=ALU.subtract)
                    nc.sync.dma_start(out=p_out[tok0:tok0 + 128, :],
                                      in_=pout_t[:])

                    # word head
                    pe = ps2.tile([128, W_EMB], F32, tag="pe")
                    pe_ops = [(h1m[k][:], proj1_w[:, k * W_EMB:(k + 1) * W_EMB])
                              for k in range(8)]
                    if proj1b is not None:
                        pe_ops.append((ones128[:], proj1b[:]))
                    for i, (l_, r_) in enumerate(pe_ops):
                        nc.tensor.matmul(pe[:, 0:W_EMB], l_, r_, start=(i == 0),
                                         stop=(i == len(pe_ops) - 1))
                    e_f = cp.tile([128, W_EMB], F32, tag="e_f")
                    nc.vector.tensor_copy(e_f[:], pe[:])
                    eT = [cp.tile([128, 128], BF, tag=f"eT{k}", name=f"eT{k}")
                          for k in range(4)]
                    for k in range(4):
                        pt = psT2.tile([128, 128], F32, tag="treT")
                        nc.tensor.transpose(out=pt[:],
                                            in_=e_f[:, k * 128:(k + 1) * 128],
                                            identity=id128[:])
                        nc.vector.tensor_copy(eT[k][:], pt[:])
                    stag = stp.tile([128, VS], BF, tag="stag")
                    zx = cp.tile([128, NVC], F32, tag="zx")
                    for v in range(NVC):
                        pv = psv.tile([128, VCW], F32, tag="pv")
                        pv_ops = [(eT[k][:],
                                   embT_w[:, k * VS + v * VCW:
                                          k * VS + (v + 1) * VCW])
                                  for k in range(4)]
                        if b2r is not None:
                            pv_ops.append((ones128[:],
                                           b2r[:, v * VCW:(v + 1) * VCW]))
                        for i, (l_, r_) in enumerate(pv_ops):
                            nc.tensor.matmul(pv[:, 0:VCW], l_, r_,
                                             start=(i == 0),
                                             stop=(i == len(pv_ops) - 1))
                        ex = cp.tile([128, VCW], F32, tag="ex")
                        nc.scalar.activation(out=ex[:], in_=pv[:], func=AF.Exp,
                                             accum_out=zx[:, v:v + 1])
                        nc.vector.tensor_copy(stag[:, v * VCW:(v + 1) * VCW],
                                              pv[:])
                    zc = cp.tile([128, 1], F32, tag="zc")
                    nc.vector.tensor_reduce(out=zc[:], in_=zx[:],
                                            axis=mybir.AxisListType.X,
                                            op=ALU.add)
                    nc.sync.dma_start(out=zin[m][:], in_=zc[:])
                    nc.gpsimd.collective_compute(
                        "AllReduce", ALU.add, replica_groups=[cores],
                        ins=[zin[m][:]], outs=[zout[m][:]])
                    stags[m] = stag

                def c_finish(m):
                    tok0 = m * 128
                    zg = cp.tile([128, 1], F32, tag="zg")
                    nc.sync.dma_start(out=zg[:], in_=zout[m][:])
                    lse = cp.tile([128, 1], F32, tag="lse")
                    nc.scalar.activation(out=lse[:], in_=zg[:], func=AF.Ln)
                    stag = stags.pop(m)
                    for v in range(NVC):
                        outv = stp.tile([128, VCW], F32, tag="outv")
                        nc.vector.tensor_scalar(
                            out=outv[:], in0=stag[:, v * VCW:(v + 1) * VCW],
                            scalar1=lse[:], scalar2=None, op0=ALU.subtract)
                        nc.sync.dma_start(
                            out=w_out[tok0:tok0 + 128, v * VCW:(v + 1) * VCW],
                            in_=outv[:])

                for m in range(nmt):
                    c_compute(m)
                    if m >= 1:
                        c_finish(m - 1)
                c_finish(nmt - 1)

    from wsplit import split_waits
    split_waits(nc)
    return nc


def _run(inputs, nsteps=T):
    d, percore, nz = _host_prep(inputs)
    nc = build(nsteps, nz)
    base = {k: np.ascontiguousarray(v) for k, v in d.items()}
    in_maps = []
    for c in range(NC):
        m = dict(base)
        m.update({k: np.ascontiguousarray(v) for k, v in percore[c].items()})
        in_maps.append(m)
    return nc, in_maps


def kernel(**inputs):
    nc, in_maps = _run(inputs)
    res = run_bass_kernel_spmd(nc, in_maps, core_ids=list(range(NC)))
    w_full = np.concatenate([res.results[c]["w_out"] for c in range(NC)], axis=1)
    p_full = res.results[0]["p_out"]
    return (p_full.reshape(T, B, POS_V).astype(np.float32),
            w_full.reshape(T, B, WORD_V).astype(np.float32))


# revision 12
# speedup vs baseline: 1.0629x; 1.0629x over previous
"""Trainium2 Bass kernel for nn_ChainPOSAwareLM (T=64, B=32, vocab 32000).

Architecture (8 NeuronCores, full I/O):
- Phase B (64-step coupled LSTM recurrence) runs REPLICATED on every core:
  per-step cross-core sync is impossible to do fast with this toolchain
  (chained ncfw collectives measure ~10-15us; remote_dma is rejected by this
  walrus build), and the ~16M recurrent weight elements cannot be re-streamed
  from HBM per step. All recurrent matmuls keep batch(32) stationary (hT as
  lhsT, produced via PE transposes) with the weight as the moving operand,
  4x column-tile packed, so the PE ingests 4*128 weight elems/cycle.
- Five large matrices are stored fp8-e4m3 so the recurrent set fits SBUF;
  psum accumulation is fp32. (numpy sim: ~3e-3 scale-relative output error.)
- h1/p_h histories stream to DRAM; phase C reloads them per 128-token m-tile.
- Phase C (tied projection + log_softmax over 32000) is vocab-sharded
  (4000/core): matmul -> exp with fused row-sum (accum_out) -> one small
  AllReduce of Z per m-tile -> out = logits - ln(Z). Pos head replicated.
"""
import numpy as np
import ml_dtypes

import concourse.bass as bass
import concourse.mybir as mybir
from concourse.tile import TileContext
from concourse.bass_utils import run_bass_kernel_spmd

BF16 = ml_dtypes.bfloat16
E4M3 = ml_dtypes.float8_e4m3fn

F32 = mybir.dt.float32
BF = mybir.dt.bfloat16
F8 = mybir.dt.float8e4
AF = mybir.ActivationFunctionType
ALU = mybir.AluOpType

T, B = 64, 32
POS_V, WORD_V = 48, 32000
P_EMB, W_EMB = 128, 512
P_HID, W_HID = 256, 1024
NC = 8
VS = WORD_V // NC
NVC = 8
VCW = VS // NVC


def _reord(w):
    i, f, g, o = np.split(w, 4, axis=0)
    return np.concatenate([i, f, o, g], axis=0)


def _host_prep(inp):
    P = {k: np.asarray(v, np.float32) for k, v in inp.items()
         if k not in ("pos", "word")}
    pos = np.asarray(inp["pos"]).astype(np.int64)
    word = np.asarray(inp["word"]).astype(np.int64)
    p_emb = P["pos_emb_W"][pos]
    w_emb = P["word_emb_W"][word]
    pre_pos = (p_emb @ _reord(P["p_Wih0"])[:, :P_EMB].T
               + _reord(P["p_bih0"][:, None])[:, 0]
               + _reord(P["p_bhh0"][:, None])[:, 0])
    pre_w0 = (w_emb @ _reord(P["w_Wih0"])[:, :W_EMB].T
              + _reord(P["w_bih0"][:, None])[:, 0]
              + _reord(P["w_bhh0"][:, None])[:, 0])
    b1 = (_reord(P["w_bih1"][:, None])[:, 0]
          + _reord(P["w_bhh1"][:, None])[:, 0])

    d = {}
    d["pre_pos"] = pre_pos.astype(BF16)                    # [T,B,1024]
    d["pre_w0"] = pre_w0.astype(BF16)                      # [T,B,4096]
    d["w2p_rhs"] = P["w2p_W"].T.astype(BF16)
    d["pwih_lw"] = _reord(P["p_Wih0"])[:, P_EMB:].T.astype(E4M3)
    d["pwhh"] = _reord(P["p_Whh0"]).T.astype(BF16)
    d["p2w_rhs"] = P["p2w_W"].T.astype(BF16)
    d["wih0_lp"] = _reord(P["w_Wih0"])[:, W_EMB:].T.astype(E4M3)
    d["whh0"] = _reord(P["w_Whh0"]).T.astype(E4M3)
    d["wih1"] = _reord(P["w_Wih1"]).T.astype(E4M3)
    d["whh1"] = _reord(P["w_Whh1"]).T.astype(E4M3)
    d["proj1_rhs"] = P["word_proj1_W"].T.astype(BF16)
    d["posproj_rhs"] = P["pos_proj_W"].T.astype(BF16)
    d["ident32"] = np.eye(32, dtype=np.float32)
    d["ident128"] = np.eye(128, dtype=np.float32)
    d["ones1"] = np.ones((1, 32), BF16)
    d["ones128r"] = np.ones((1, 128), BF16)
    # biases that are not folded into precomps
    d["w2p_b_row"] = P["w2p_b"][None, :].astype(BF16)
    d["p2w_b_row"] = P["p2w_b"][None, :].astype(BF16)
    d["b1_row"] = b1[None, :].astype(BF16)
    d["proj1_b_row"] = P["word_proj1_b"][None, :].astype(BF16)
    d["posproj_b_row"] = P["pos_proj_b"][None, :].astype(BF16)
    nz = {
        "w2p_b_row": np.abs(P["w2p_b"]).max() > 0,
        "p2w_b_row": np.abs(P["p2w_b"]).max() > 0,
        "b1_row": np.abs(b1).max() > 0,
        "proj1_b_row": np.abs(P["word_proj1_b"]).max() > 0,
        "posproj_b_row": np.abs(P["pos_proj_b"]).max() > 0,
        "b2_row": np.abs(P["word_proj2_b"]).max() > 0,
    }
    embT = P["word_emb_W"].T.astype(BF16)
    b2 = P["word_proj2_b"].astype(BF16)
    percore = []
    for c in range(NC):
        percore.append({
            "embT_sl": np.ascontiguousarray(embT[:, c * VS:(c + 1) * VS]),
            "b2_row": np.ascontiguousarray(b2[None, c * VS:(c + 1) * VS]),
        })
    return d, percore, nz


def build(nsteps, nz):
    nc = bass.Bass()
    q = {}

    def par(name, shape, dt):
        q[name] = nc.declare_dram_parameter(name, list(shape), dt, isOutput=False)

    par("pre_pos", (T, B, 4 * P_HID), BF)
    par("pre_w0", (T, B, 4 * W_HID), BF)
    par("w2p_rhs", (W_HID, W_HID), BF)
    par("pwih_lw", (W_HID, 4 * P_HID), F8)
    par("pwhh", (P_HID, 4 * P_HID), BF)
    par("p2w_rhs", (P_HID, P_HID), BF)
    par("wih0_lp", (P_HID, 4 * W_HID), F8)
    par("whh0", (W_HID, 4 * W_HID), F8)
    par("wih1", (W_HID, 4 * W_HID), F8)
    par("whh1", (W_HID, 4 * W_HID), F8)
    par("proj1_rhs", (W_HID, W_EMB), BF)
    par("posproj_rhs", (P_HID, POS_V), BF)
    par("ident32", (32, 32), F32)
    par("ident128", (128, 128), F32)
    par("ones1", (1, 32), BF)
    par("ones128r", (1, 128), BF)
    par("w2p_b_row", (1, W_HID), BF)
    par("p2w_b_row", (1, P_HID), BF)
    par("b1_row", (1, 4 * W_HID), BF)
    par("proj1_b_row", (1, W_EMB), BF)
    par("posproj_b_row", (1, POS_V), BF)
    par("embT_sl", (W_EMB, VS), BF)
    par("b2_row", (1, VS), BF)

    ntok = nsteps * B
    nmt = ntok // 128
    w_out = nc.declare_dram_parameter("w_out", [ntok, VS], F32, isOutput=True)
    p_out = nc.declare_dram_parameter("p_out", [ntok, POS_V], F32, isOutput=True)

    h1_dram = nc.dram_tensor("h1_hist", [128, nsteps, 8, 32], BF)
    ph_dram = nc.dram_tensor("ph_hist", [128, nsteps, 2, 32], BF)
    zin = [nc.dram_tensor(f"zin{m}", [128, 1], F32) for m in range(nmt)]
    zout = [nc.dram_tensor(f"zout{m}", [128, 1], F32, addr_space="Shared")
            for m in range(nmt)]
    cores = list(range(NC))

    with TileContext(nc) as tc:
        with tc.tile_pool(name="const", bufs=1) as cst:
            def loadc(name):
                src = q[name]
                t_ = cst.tile(list(src.shape), src.dtype, tag=name)
                nc.sync.dma_start(out=t_[:], in_=src[:])
                return t_

            id32 = loadc("ident32")
            id128 = loadc("ident128")
            ones1 = loadc("ones1")
            ones128 = loadc("ones128r")

            # ===================== PHASE B =====================
            with (
                tc.tile_pool(name="bw", bufs=1) as wp,
                tc.tile_pool(name="bstate", bufs=1) as sp,
                tc.tile_pool(name="bwork", bufs=1) as kp,
                tc.tile_pool(name="bpipe1", bufs=1) as dp1,
                tc.tile_pool(name="bps", bufs=1, space="PSUM") as ps,
                tc.tile_pool(name="bpsg", bufs=2, space="PSUM") as psg,
                tc.tile_pool(name="bpsT", bufs=2, space="PSUM") as psT,
            ):
                def load_kn(name, K, N):
                    src = q[name]
                    nk = K // 128
                    t_ = wp.tile([128, nk * N], src.dtype, tag=name)
                    for k in range(nk):
                        nc.sync.dma_start(out=t_[:, k * N:(k + 1) * N],
                                          in_=src[k * 128:(k + 1) * 128, :])
                    return t_

                id32b = cst.tile([32, 32], BF, tag="id32b")
                nc.vector.tensor_copy(id32b[:], id32[:])
                w2p_w = load_kn("w2p_rhs", W_HID, W_HID)
                pwihlw_w = load_kn("pwih_lw", W_HID, 4 * P_HID)
                pwhh_w = load_kn("pwhh", P_HID, 4 * P_HID)
                p2w_w = load_kn("p2w_rhs", P_HID, P_HID)
                wih0lp_w = load_kn("wih0_lp", P_HID, 4 * W_HID)
                whh0_w = load_kn("whh0", W_HID, 4 * W_HID)
                wih1_w = load_kn("wih1", W_HID, 4 * W_HID)
                whh1_w = load_kn("whh1", W_HID, 4 * W_HID)
                w2pb = loadc("w2p_b_row") if nz["w2p_b_row"] else None
                p2wb = loadc("p2w_b_row") if nz["p2w_b_row"] else None
                b1r = loadc("b1_row") if nz["b1_row"] else None

                # state windows (bf16 transposed layout), double buffered
                h1w = [sp.tile([128, 8 * 32], BF, tag=f"h1w{j}", name=f"h1w{j}") for j in range(2)]
                phw = [sp.tile([128, 2 * 32], BF, tag=f"phw{j}", name=f"phw{j}") for j in range(2)]
                h0w = [sp.tile([128, 8 * 32], BF, tag=f"h0w{j}", name=f"h0w{j}") for j in range(2)]
                nc.vector.memset(h1w[1][:], 0.0)
                nc.vector.memset(phw[1][:], 0.0)
                nc.vector.memset(h0w[1][:], 0.0)
                c_pos = sp.tile([64, P_HID], F32, tag="c_pos")
                c_w0 = sp.tile([64, W_HID], F32, tag="c_w0")
                c_w1 = sp.tile([64, W_HID], F32, tag="c_w1")
                nc.vector.memset(c_pos[32:64, :], 0.0)
                nc.vector.memset(c_w0[32:64, :], 0.0)
                nc.vector.memset(c_w1[32:64, :], 0.0)

                def packed(psum, nper, kblocks, pre=None, bias=None):
                    """col-packed gate accumulation into psum [128, nper].
                    kblocks: (lhsT_ap, w_tile, kidx, wN). Issue order is
                    (sub, k, cg) so 4 consecutive matmuls hit the 4 column
                    groups and stream concurrently."""
                    nsub = (nper + 511) // 512
                    ops = list(kblocks)
                    extra = []
                    if bias is not None:
                        extra.append(("bias", bias))
                    if pre is not None:
                        extra.append(("pre", pre))
                    nops = len(ops) + len(extra)
                    for sub in range(nsub):
                        c0 = sub * 512
                        cw = min(512, nper - c0)
                        for i in range(nops):
                            for cg in range(4):
                                base = cg * nper + c0
                                if i < len(ops):
                                    lhsT, w_t, kidx, wN = ops[i]
                                    l_ = lhsT
                                    r_ = w_t[:, kidx * wN + base:
                                             kidx * wN + base + cw]
                                else:
                                    kind, tile_ = extra[i - len(ops)]
                                    l_ = ones1[:] if kind == "bias" else id32b[:]
                                    r_ = tile_[:, base:base + cw]
                                nc.tensor.matmul(
                                    psum[cg * 32:(cg + 1) * 32, c0:c0 + cw],
                                    l_, r_,
                                    start=(i == 0), stop=(i == nops - 1),
                                    tile_position=(0, cg * 32))


                def cell(psum, nper, c_tile, hout, cellid=""):
                    sifo = kp.tile([96, nper], BF, tag=f"sifo{cellid}",
                                   name=f"sifo{cellid}")
                    tgtc = kp.tile([96, nper], BF, tag=f"tgtc{cellid}",
                                   name=f"tgtc{cellid}")
                    t1 = kp.tile([64, nper], BF, tag=f"t1{cellid}",
                                 name=f"t1{cellid}")
                    t2 = kp.tile([64, nper], F32, tag=f"t2{cellid}",
                                 name=f"t2{cellid}")
                    nc.scalar.activation(out=sifo[0:96, :nper],
                                         in_=psum[0:96, :nper], func=AF.Sigmoid)
                    nc.gpsimd.tensor_tensor(out=t2[32:64, :nper],
                                            in0=sifo[32:64, :nper],
                                            in1=c_tile[32:64, :nper], op=ALU.mult)
                    nc.scalar.activation(out=tgtc[0:32, :nper],
                                         in_=psum[96:128, :nper], func=AF.Tanh)
                    nc.vector.tensor_tensor(out=t1[32:64, :nper],
                                            in0=sifo[0:32, :nper],
                                            in1=tgtc[0:32, :nper], op=ALU.mult)
                    nc.vector.tensor_tensor(out=c_tile[32:64, :nper],
                                            in0=t1[32:64, :nper],
                                            in1=t2[32:64, :nper], op=ALU.add)
                    nc.scalar.activation(out=tgtc[64:96, :nper],
                                         in_=c_tile[32:64, :nper], func=AF.Tanh)
                    nc.vector.tensor_tensor(out=hout[0:32, :nper],
                                            in0=sifo[64:96, :nper],
                                            in1=tgtc[64:96, :nper], op=ALU.mult)

                def transp(h_sbuf, nper, dst, dst_base):
                    """h_sbuf [32, nper] bf16 -> bf16 transposed into
                    dst[:, dst_base + k*32 ...]."""
                    for k in range(nper // 128):
                        pt = psT.tile([128, 32], BF, tag="tr")
                        nc.tensor.transpose(
                            out=pt[:], in_=h_sbuf[0:32, k * 128:(k + 1) * 128],
                            identity=id32b[:])
                        nc.vector.tensor_copy(
                            dst[:, dst_base + k * 32:dst_base + (k + 1) * 32],
                            pt[:])

                for t in range(nsteps):
                    cur, prv = t % 2, (t + 1) % 2
                    # ---- last_w = tanh(w2p @ h1_prev (+b)) ----
                    pw = psg.tile([128, 256], F32, tag="pgate")
                    packed(pw, 256,
                           [(h1w[prv][:, k * 32:(k + 1) * 32], w2p_w, k, W_HID)
                            for k in range(8)], bias=w2pb)
                    lastw = kp.tile([32, W_HID], BF, tag="lastw")
                    for cg in range(4):
                        nc.scalar.activation(
                            out=lastw[0:32, cg * 256:(cg + 1) * 256],
                            in_=pw[cg * 32:(cg + 1) * 32, 0:256], func=AF.Tanh)
                    lwT = kp.tile([128, 8 * 32], BF, tag="lwT")
                    transp(lastw, W_HID, lwT, 0)

                    # ---- pos gates / cell ----
                    pre_p = dp1.tile([32, 4 * P_HID], BF, tag="pre_p")
                    nc.sync.dma_start(out=pre_p[:], in_=q["pre_pos"][t])
                    pg = psg.tile([128, 256], F32, tag="pgate")
                    packed(pg, 256,
                           [(phw[prv][:, k * 32:(k + 1) * 32], pwhh_w, k,
                             4 * P_HID) for k in range(2)]
                           + [(lwT[:, k * 32:(k + 1) * 32], pwihlw_w, k,
                               4 * P_HID) for k in range(8)],
                           pre=pre_p)
                    ph_new = kp.tile([32, P_HID], BF, tag="phnew")
                    cell(pg, P_HID, c_pos, ph_new, "p")
                    transp(ph_new, P_HID, phw[cur], 0)
                    nc.sync.dma_start(out=ph_dram[:, t], in_=phw[cur][:])

                    # ---- last_p = tanh(p2w @ p_h (+b)) ----
                    pp = psg.tile([32, P_HID], F32, tag="pgate")
                    lp_ops = [(phw[cur][:, k * 32:(k + 1) * 32],
                               p2w_w[:, k * P_HID:(k + 1) * P_HID])
                              for k in range(2)]
                    if p2wb is not None:
                        lp_ops.append((ones1[:], p2wb[:]))
                    for i, (l_, r_) in enumerate(lp_ops):
                        nc.tensor.matmul(pp[:, 0:P_HID], l_, r_,
                                         start=(i == 0),
                                         stop=(i == len(lp_ops) - 1))
                    lastp = kp.tile([32, P_HID], BF, tag="lastp")
                    nc.scalar.activation(out=lastp[:], in_=pp[:], func=AF.Tanh)
                    lpT = kp.tile([128, 2 * 32], BF, tag="lpT")
                    transp(lastp, P_HID, lpT, 0)

                    # ---- word LSTM 0 ----
                    pre0 = dp1.tile([32, 4 * W_HID], BF, tag="pre0")
                    nc.sync.dma_start(out=pre0[:], in_=q["pre_w0"][t])
                    g0 = ps.tile([128, W_HID], F32, tag="g0")
                    packed(g0, W_HID,
                           [(h0w[prv][:, k * 32:(k + 1) * 32], whh0_w, k,
                             4 * W_HID) for k in range(8)]
                           + [(lpT[:, k * 32:(k + 1) * 32], wih0lp_w, k,
                               4 * W_HID) for k in range(2)],
                           pre=pre0)
                    h0_new = kp.tile([32, W_HID], BF, tag="h0new")
                    cell(g0, W_HID, c_w0, h0_new, "0")
                    transp(h0_new, W_HID, h0w[cur], 0)

                    # ---- word LSTM 1 ----
                    g1 = ps.tile([128, W_HID], F32, tag="g1")
                    packed(g1, W_HID,
                           [(h1w[prv][:, k * 32:(k + 1) * 32], whh1_w, k,
                             4 * W_HID) for k in range(8)]
                           + [(h0w[cur][:, k * 32:(k + 1) * 32], wih1_w, k,
                               4 * W_HID) for k in range(8)],
                           bias=b1r)
                    h1_new = kp.tile([32, W_HID], BF, tag="h1new")
                    cell(g1, W_HID, c_w1, h1_new, "1")
                    transp(h1_new, W_HID, h1w[cur], 0)
                    nc.sync.dma_start(out=h1_dram[:, t], in_=h1w[cur][:])

            # ===================== PHASE C =====================
            with (
                tc.tile_pool(name="cw", bufs=1) as cw,
                tc.tile_pool(name="cwork", bufs=2) as cp,
                tc.tile_pool(name="cstage", bufs=3) as stp,
                tc.tile_pool(name="cps", bufs=1, space="PSUM") as ps2,
                tc.tile_pool(name="cpsv", bufs=2, space="PSUM") as psv,
                tc.tile_pool(name="cpsT", bufs=2, space="PSUM") as psT2,
            ):
                def load_kn2(name, K, N):
                    src = q[name]
                    nk = K // 128
                    t_ = cw.tile([128, nk * N], src.dtype, tag=name)
                    for k in range(nk):
                        nc.sync.dma_start(out=t_[:, k * N:(k + 1) * N],
                                          in_=src[k * 128:(k + 1) * 128, :])
                    return t_

                proj1_w = load_kn2("proj1_rhs", W_HID, W_EMB)
                posproj_w = load_kn2("posproj_rhs", P_HID, POS_V)
                embT_w = load_kn2("embT_sl", W_EMB, VS)
                proj1b = loadc("proj1_b_row") if nz["proj1_b_row"] else None
                posprojb = loadc("posproj_b_row") if nz["posproj_b_row"] else None
                b2r = loadc("b2_row") if nz["b2_row"] else None

                stags = {}
                lses = {}

                def c_compute(m):
                    tok0 = m * 128
                    s0 = tok0 // 32
                    h1m = [cp.tile([128, 128], BF, tag=f"h1m{k}", name=f"h1m{k}")
                           for k in range(8)]
                    for k in range(8):
                        nc.sync.dma_start(
                            out=h1m[k].rearrange("p (s c) -> p s c", s=4),
                            in_=h1_dram[:, s0:s0 + 4, k, :])
                    phm = [cp.tile([128, 128], BF, tag=f"phm{k}", name=f"phm{k}")
                           for k in range(2)]
                    for k in range(2):
                        nc.sync.dma_start(
                            out=phm[k].rearrange("p (s c) -> p s c", s=4),
                            in_=ph_dram[:, s0:s0 + 4, k, :])

                    # pos head
                    pl = ps2.tile([128, POS_V], F32, tag="pl")
                    pl_ops = [(phm[k][:], posproj_w[:, k * POS_V:(k + 1) * POS_V])
                              for k in range(2)]
                    if posprojb is not None:
                        pl_ops.append((ones128[:], posprojb[:]))
                    for i, (l_, r_) in enumerate(pl_ops):
                        nc.tensor.matmul(pl[:, 0:POS_V], l_, r_, start=(i == 0),
                                         stop=(i == len(pl_ops) - 1))
                    pexp = cp.tile([128, POS_V], F32, tag="pexp")
                    pzs = cp.tile([128, 1], F32, tag="pzs")
                    nc.scalar.activation(out=pexp[:], in_=pl[:], func=AF.Exp,
                                         accum_out=pzs[:])
                    plse = cp.tile([128, 1], F32, tag="plse")
                    nc.scalar.activation(out=plse[:], in_=pzs[:], func=AF.Ln)
                    pout_t = cp.tile([128, POS_V], F32, tag="pout_t")
                    nc.vector.tensor_scalar(out=pout_t[:], in0=pl[:],
                                            scalar1=plse[:], scalar2=None,
                                            op0=ALU.subtract)
                    nc.sync.dma_start(out=p_out[tok0:tok0 + 128, :],
                                      in_=pout_t[:])

                    # word head
                    pe = ps2.tile([128, W_EMB], F32, tag="pe")
                    pe_ops = [(h1m[k][:], proj1_w[:, k * W_EMB:(k + 1) * W_EMB])
                              for k in range(8)]
                    if proj1b is not None:
                        pe_ops.append((ones128[:], proj1b[:]))
                    for i, (l_, r_) in enumerate(pe_ops):
                        nc.tensor.matmul(pe[:, 0:W_EMB], l_, r_, start=(i == 0),
                                         stop=(i == len(pe_ops) - 1))
                    e_f = cp.tile([128, W_EMB], F32, tag="e_f")
                    nc.vector.tensor_copy(e_f[:], pe[:])
                    eT = [cp.tile([128, 128], BF, tag=f"eT{k}", name=f"eT{k}")
                          for k in range(4)]
                    for k in range(4):
                        pt = psT2.tile([128, 128], F32, tag="treT")
                        nc.tensor.transpose(out=pt[:],
                                            in_=e_f[:, k * 128:(k + 1) * 128],
                                            identity=id128[:])
                        nc.vector.tensor_copy(eT[k][:], pt[:])
                    stag = stp.tile([128, VS], BF, tag="stag")
                    zx = cp.tile([128, NVC], F32, tag="zx")
                    for v in range(NVC):
                        pv = psv.tile([128, VCW], F32, tag="pv")
                        pv_ops = [(eT[k][:],
                                   embT_w[:, k * VS + v * VCW:
                                          k * VS + (v + 1) * VCW])
                                  for k in range(4)]
                        if b2r is not None:
                            pv_ops.append((ones128[:],
                                           b2r[:, v * VCW:(v + 1) * VCW]))
                        for i, (l_, r_) in enumerate(pv_ops):
                            nc.tensor.matmul(pv[:, 0:VCW], l_, r_,
                                             start=(i == 0),
                                             stop=(i == len(pv_ops) - 1))
                        ex = cp.tile([128, VCW], F32, tag="ex")
                        nc.scalar.activation(out=ex[:], in_=pv[:], func=AF.Exp,
                                             accum_out=zx[:, v:v + 1])
                        nc.vector.tensor_copy(stag[:, v * VCW:(v + 1) * VCW],
                                              pv[:])
                    zc = cp.tile([128, 1], F32, tag="zc")
                    nc.vector.tensor_reduce(out=zc[:], in_=zx[:],
                                            axis=mybir.AxisListType.X,
                                            op=ALU.add)
                    nc.sync.dma_start(out=zin[m][:], in_=zc[:])
                    nc.gpsimd.collective_compute(
                        "AllReduce", ALU.add, replica_groups=[cores],
                        ins=[zin[m][:]], outs=[zout[m][:]])
                    stags[m] = stag

                def c_finish(m):
                    tok0 = m * 128
                    zg = cp.tile([128, 1], F32, tag="zg")
                    nc.sync.dma_start(out=zg[:], in_=zout[m][:])
                    lse = cp.tile([128, 1], F32, tag="lse")
                    nc.scalar.activation(out=lse[:], in_=zg[:], func=AF.Ln)
                    stag = stags.pop(m)
                    for v in range(NVC):
                        outv = stp.tile([128, VCW], F32, tag="outv")
                        nc.vector.tensor_scalar(
                            out=outv[:], in0=stag[:, v * VCW:(v + 1) * VCW],
                            scalar1=lse[:], scalar2=None, op0=ALU.subtract)
                        nc.sync.dma_start(
                            out=w_out[tok0:tok0 + 128, v * VCW:(v + 1) * VCW],
                            in_=outv[:])

                for m in range(nmt):
                    c_compute(m)
                    if m >= 1:
                        c_finish(m - 1)
                c_finish(nmt - 1)

    from wsplit import split_waits
    split_waits(nc)
    return nc


def _run(inputs, nsteps=T):
    d, percore, nz = _host_prep(inputs)
    nc = build(nsteps, nz)
    base = {k: np.ascontiguousarray(v) for k, v in d.items()}
    in_maps = []
    for c in range(NC):
        m = dict(base)
        m.update({k: np.ascontiguousarray(v) for k, v in percore[c].items()})
        in_maps.append(m)
    return nc, in_maps


def kernel(**inputs):
    nc, in_maps = _run(inputs)
    res = run_bass_kernel_spmd(nc, in_maps, core_ids=list(range(NC)))
    w_full = np.concatenate([res.results[c]["w_out"] for c in range(NC)], axis=1)
    p_full = res.results[0]["p_out"]
    return (p_full.reshape(T, B, POS_V).astype(np.float32),
            w_full.reshape(T, B, WORD_V).astype(np.float32))
